# revision 1
# baseline (speedup 1.0000x reference)
"""Trainium2 Bass kernel for AttentionWithSharedWeights (LoRA attention, GQA, RoPE).

Sharding over 8 NeuronCores: batch (4) x head-group (2).  Each core computes
8 Q heads / 2 KV heads of one batch and a partial (head-sliced) output
projection; the host sums the two partials per batch.

Host-side preprocessing (inside kernel(), pure layout/reparameterization):
  - LoRA folded into dense weights (W_eff = W + B @ A)
  - x transposed per batch; weights transposed so every matmul operand is in
    its natural [contraction-dim-major] layout (no on-chip transposes)
  - RoPE rows pre-permuted (even dims then odd dims per head) so the rotation
    becomes a constant 128x128 matmul + two elementwise multiplies
  - cos/sin tables, causal masks, all-ones matrix precomputed

Device program (single SPMD program, all matmuls in float32r = full PE rate):
  A) QKV projections from resident weights + streamed x.T chunks, fused RoPE;
     K (feature-major) and V (token-major) stay resident in SBUF, Q spills to
     DRAM.
  B) Causal attention in scores-transposed layout: S^T = K^T_tile.T @ Q^T,
     exp on the scalar engine (softmax scale folded in, two k-tiles per
     activation), P@V accumulated in PSUM per 512-token q-chunk, denominator
     accumulated as allones @ E in a parallel PSUM bank, then one reciprocal
     + multiply normalizes during PSUM->SBUF eviction.  Diagonal tiles only
     compute their causal q-range.
  C) Output projection from resident wo, token-major y written straight out.
"""

import numpy as np

B, S, DIM = 4, 2048, 2048
NH, NKV, HD = 16, 4, 128
LR = 16          # lora rank
SC = 512         # sequence chunk
NSC = S // SC    # 4
NKT = S // HD    # 16 k-tiles
HPC = NH // 2    # 8 q heads per core
KVPC = NKV // 2  # 2 kv heads per core
FQ = HPC * HD    # 1024 q features per core
FKV = KVPC * HD  # 256 kv features per core
SCALE = 1.0 / float(np.sqrt(HD))

_cache = {}


def _build_program():
    import concourse.mybir as mybir
    import concourse.tile as tile
    from concourse import bacc

    f32 = mybir.dt.float32
    f32r = mybir.dt.float32r
    Exp = mybir.ActivationFunctionType.Exp

    nc = bacc.Bacc()

    # ---- DRAM parameters (per-core views, host-prepared layouts) ----
    xt_d = nc.declare_dram_parameter("xt", [DIM, S], f32r, isOutput=False)
    wq_d = nc.declare_dram_parameter("wq", [DIM, FQ], f32r, isOutput=False)
    wk_d = nc.declare_dram_parameter("wk", [DIM, FKV], f32r, isOutput=False)
    wv_d = nc.declare_dram_parameter("wv", [DIM, FKV], f32r, isOutput=False)
    wo_d = nc.declare_dram_parameter("wo", [FQ, DIM], f32r, isOutput=False)
    cs_d = nc.declare_dram_parameter("cs", [HD, S], f32, isOutput=False)
    sn_d = nc.declare_dram_parameter("sn", [HD, S], f32, isOutput=False)
    rt_d = nc.declare_dram_parameter("rt", [HD, HD], f32r, isOutput=False)
    on_d = nc.declare_dram_parameter("on", [HD, HD], f32r, isOutput=False)
    mk_d = nc.declare_dram_parameter("mk", [HD, 4, SC], f32, isOutput=False)
    y_d = nc.declare_dram_parameter("y", [S, DIM], f32, isOutput=True)

    # internal spills, split per sequence chunk so cross-phase DRAM
    # dependencies are per-chunk rather than whole-tensor
    qt_ds = [nc.dram_tensor(f"qt_spill{i}", [FQ, SC], f32r) for i in range(NSC)]
    ot_ds = [nc.dram_tensor(f"ot_spill{i}", [FQ, SC], f32r) for i in range(NSC)]

    with tile.TileContext(nc) as tc:
        # K/V stay in SBUF across phases A and B: raw allocations so that
        # pool stacks of each phase bump above them without overlap.
        kt_t = nc.alloc_sbuf_tensor("kt_res", [HD, KVPC, S], f32r)
        v_t = nc.alloc_sbuf_tensor("v_res", [HD, NKT, FKV], f32r)
        kt_sb = kt_t[:]       # K feat-major, roped
        v_sb = v_t[:]         # V token-major
        if True:

            # ---------------- Phase A: projections + RoPE ----------------
            with tc.tile_pool(name="pa_w", bufs=1) as pw, \
                 tc.tile_pool(name="pa_x", bufs=1) as px, \
                 tc.tile_pool(name="pa_r", bufs=2) as pr, \
                 tc.tile_pool(name="pa_ps", bufs=4, space="PSUM") as pps, \
                 tc.tile_pool(name="pa_rot", bufs=2, space="PSUM") as prot, \
                 tc.tile_pool(name="pa_vps", bufs=2, space="PSUM") as pvps:

                wq_sb = pw.tile([HD, NKT, FQ], f32r)
                wk_sb = pw.tile([HD, NKT, FKV], f32r)
                wv_sb = pw.tile([HD, NKT, FKV], f32r)
                rt_sb = pw.tile([HD, HD], f32r)

                for ft in range(HPC):
                    nc.sync.dma_start(
                        wq_sb[:, :, ft * HD:(ft + 1) * HD],
                        wq_d[:, ft * HD:(ft + 1) * HD].rearrange(
                            "(k p) f -> p k f", p=HD))
                nc.sync.dma_start(wk_sb[:], wk_d[:].rearrange("(k p) f -> p k f", p=HD))
                nc.sync.dma_start(wv_sb[:], wv_d[:].rearrange("(k p) f -> p k f", p=HD))
                nc.sync.dma_start(rt_sb[:], rt_d[:])

                def rope_block(raw_ps, fpool, cs_sl, sn_sl, out_ap):
                    """raw_ps: PSUM [128, SC] pre-rope; writes roped f32r to out_ap."""
                    raw = fpool.tile([HD, SC], f32r, tag="rope_raw")
                    nc.any.tensor_copy(out=raw[:], in_=raw_ps[:])
                    rot_ps = prot.tile([HD, SC], f32)
                    nc.tensor.matmul(rot_ps[:], rt_sb[:], raw[:],
                                     start=True, stop=True)
                    tmp = fpool.tile([HD, SC], f32, tag="rope_tmp")
                    nc.vector.tensor_mul(tmp[:], raw[:].bitcast(f32), cs_sl)
                    e1 = fpool.tile([HD, SC], f32, tag="rope_e1")
                    nc.vector.tensor_mul(e1[:], rot_ps[:], sn_sl)
                    nc.vector.tensor_add(out_ap, tmp[:], e1[:])

                for sc in range(NSC):
                    ssl = slice(sc * SC, (sc + 1) * SC)
                    xc = px.tile([HD, NKT, SC], f32r, tag="xc")
                    cs_sb = px.tile([HD, SC], f32, tag="cs")
                    sn_sb = px.tile([HD, SC], f32, tag="sn")
                    nc.sync.dma_start(cs_sb[:], cs_d[:, ssl])
                    nc.sync.dma_start(sn_sb[:], sn_d[:, ssl])
                    for kt in range(NKT):
                        nc.sync.dma_start(
                            xc[:, kt, :], xt_d[kt * HD:(kt + 1) * HD, ssl])

                    def xck(kt):
                        return xc[:, kt, :]

                    # Q projection + rope -> spill to DRAM
                    for ft in range(HPC):
                        fsl = slice(ft * HD, (ft + 1) * HD)
                        q_ps = pps.tile([HD, SC], f32, tag="qk_ps")
                        for kt in range(NKT):
                            nc.tensor.matmul(q_ps[:], wq_sb[:, kt, fsl], xck(kt),
                                             start=(kt == 0), stop=(kt == NKT - 1))
                        qfin = pr.tile([HD, SC], f32r, tag="qfin")
                        rope_block(q_ps, pr, cs_sb[:], sn_sb[:], qfin[:])
                        nc.sync.dma_start(qt_ds[sc][fsl, :], qfin[:])

                    # K projection + rope -> resident SBUF
                    for ft in range(KVPC):
                        fsl = slice(ft * HD, (ft + 1) * HD)
                        k_ps = pps.tile([HD, SC], f32, tag="qk_ps")
                        for kt in range(NKT):
                            nc.tensor.matmul(k_ps[:], wk_sb[:, kt, fsl], xck(kt),
                                             start=(kt == 0), stop=(kt == NKT - 1))
                        rope_block(k_ps, pr, cs_sb[:], sn_sb[:],
                                   kt_sb[:, ft, ssl])

                    # V projection, token-major -> resident SBUF
                    for st in range(SC // HD):
                        tsl = slice(st * HD, (st + 1) * HD)
                        v_ps = pvps.tile([HD, FKV], f32, tag="v_ps")
                        for kt in range(NKT):
                            nc.tensor.matmul(v_ps[:], xck(kt)[:, tsl], wv_sb[:, kt, :],
                                             start=(kt == 0), stop=(kt == NKT - 1))
                        nc.any.tensor_copy(
                            out=v_sb[:, sc * (SC // HD) + st, :], in_=v_ps[:])

        # Phase C weights: pool opened before phase B so the wo load (8.4 MB)
        # overlaps attention compute.
        with tc.tile_pool(name="pc_w", bufs=1) as pcw:
            wo_sb = pcw.tile([HD, HPC, DIM], f32r)

            # ---------------- Phase B: causal attention ----------------
            with tc.tile_pool(name="pb_c", bufs=1) as pbc, \
                 tc.tile_pool(name="pb_q", bufs=3) as pbq, \
                 tc.tile_pool(name="pb_e", bufs=8) as pbe, \
                 tc.tile_pool(name="pb_o", bufs=3) as pbo, \
                 tc.tile_pool(name="pb_sps", bufs=2, space="PSUM") as sps, \
                 tc.tile_pool(name="pb_ops", bufs=3, space="PSUM") as ops, \
                 tc.tile_pool(name="pb_bps", bufs=1, space="PSUM") as bps:

                on_sb = pbc.tile([HD, HD], f32r)
                mk_sb = pbc.tile([HD, 4, SC], f32)
                nc.sync.dma_start(on_sb[:], on_d[:])
                nc.sync.dma_start(mk_sb[:], mk_d[:])

                for h in range(HPC):
                    kv = h // (HPC // KVPC)
                    # spread the phase-C wo load across phase B
                    nc.sync.dma_start(
                        wo_sb[:, h, :], wo_d[h * HD:(h + 1) * HD, :])
                    for qc in range(NSC):
                        qsl = slice(qc * SC, (qc + 1) * SC)
                        qt = pbq.tile([HD, SC], f32r, tag="qt")
                        nc.sync.dma_start(
                            qt[:], qt_ds[qc][h * HD:(h + 1) * HD, :])
                        ot_ps = ops.tile([HD, SC], f32, tag="ot_ps")
                        bc_ps = bps.tile([HD, SC], f32, tag="bc_ps")
                        nkt = 4 * qc + 4
                        for kp in range(nkt // 2):
                            kt0, kt1 = 2 * kp, 2 * kp + 1
                            s_ps = sps.tile([HD, 2, SC], f32, tag="s_ps")
                            if kt1 < 4 * qc:
                                # off-diagonal pair: full width, one exp
                                for i, kt in enumerate((kt0, kt1)):
                                    nc.tensor.matmul(
                                        s_ps[:, i, :],
                                        kt_sb[:, kv, kt * HD:(kt + 1) * HD],
                                        qt[:], start=True, stop=True)
                                e = pbe.tile([HD, 2, SC], f32r, tag="e")
                                nc.scalar.activation(e[:], s_ps[:], Exp,
                                                     scale=SCALE)
                                for i, kt in enumerate((kt0, kt1)):
                                    nc.tensor.matmul(
                                        ot_ps[:],
                                        v_sb[:, kt, kv * HD:(kv + 1) * HD],
                                        e[:, i, :], start=(kt == 0),
                                        stop=(kt == nkt - 1),
                                        skip_group_check=True)
                                    nc.tensor.matmul(
                                        bc_ps[:], on_sb[:], e[:, i, :],
                                        start=(kt == 0), stop=(kt == nkt - 1),
                                        skip_group_check=True)
                            else:
                                # diagonal pair: only the causal q-range
                                # [128*r, 512) of each k-tile is live
                                e = pbe.tile([HD, 2, SC], f32r, tag="e")
                                for i, kt in enumerate((kt0, kt1)):
                                    r = kt - 4 * qc
                                    q0 = r * HD
                                    nc.tensor.matmul(
                                        s_ps[:, i, q0:],
                                        kt_sb[:, kv, kt * HD:(kt + 1) * HD],
                                        qt[:, q0:], start=True, stop=True)
                                    nc.scalar.activation(
                                        e[:, i, q0:], s_ps[:, i, q0:], Exp,
                                        scale=SCALE)
                                    # intra-tile triangle mask (in place)
                                    nc.vector.tensor_mul(
                                        e[:, i, q0:q0 + HD],
                                        e[:, i, q0:q0 + HD].bitcast(f32),
                                        mk_sb[:, 0, 0:HD])
                                    nc.tensor.matmul(
                                        ot_ps[:, q0:],
                                        v_sb[:, kt, kv * HD:(kv + 1) * HD],
                                        e[:, i, q0:], start=(kt == 0),
                                        stop=(kt == nkt - 1),
                                        skip_group_check=True)
                                    nc.tensor.matmul(
                                        bc_ps[:, q0:], on_sb[:], e[:, i, q0:],
                                        start=(kt == 0), stop=(kt == nkt - 1),
                                        skip_group_check=True)
                        inv = pbo.tile([HD, SC], f32, tag="inv")
                        nc.vector.reciprocal(inv[:], bc_ps[:])
                        ot_sb = pbo.tile([HD, SC], f32r, tag="ot_sb")
                        nc.vector.tensor_mul(ot_sb[:], ot_ps[:], inv[:])
                        nc.sync.dma_start(
                            ot_ds[qc][h * HD:(h + 1) * HD, :], ot_sb[:])

            # ---------------- Phase C: output projection + LoRA ----------------
            with tc.tile_pool(name="pc_o", bufs=3) as pco, \
                 tc.tile_pool(name="pc_y", bufs=4) as pcy, \
                 tc.tile_pool(name="pc_yps", bufs=4, space="PSUM") as yps:

                for qc in range(NSC):
                    qsl = slice(qc * SC, (qc + 1) * SC)
                    otq = pco.tile([HD, HPC, SC], f32r, tag="otq")
                    for ft in range(HPC):
                        nc.sync.dma_start(
                            otq[:, ft, :], ot_ds[qc][ft * HD:(ft + 1) * HD, :])
                    # main projection
                    for dc in range(DIM // SC):
                        dsl = slice(dc * SC, (dc + 1) * SC)
                        for qs in range(SC // HD):
                            qt0 = qc * SC + qs * HD
                            y_ps = yps.tile([HD, SC], f32, tag="y_ps")
                            for ft in range(HPC):
                                nc.tensor.matmul(
                                    y_ps[:], otq[:, ft, qs * HD:(qs + 1) * HD],
                                    wo_sb[:, ft, dsl],
                                    start=(ft == 0), stop=(ft == HPC - 1))
                            y_sb = pcy.tile([HD, SC], f32, tag="y_sb")
                            nc.any.tensor_copy(out=y_sb[:], in_=y_ps[:])
                            nc.sync.dma_start(y_d[qt0:qt0 + HD, dsl], y_sb[:])

    nc.finalize()
    return nc


def _rope_perm(nheads):
    """Row permutation putting even dims first within each head."""
    idx = []
    for h in range(nheads):
        base = h * HD
        idx.extend(base + 2 * j for j in range(HD // 2))
        idx.extend(base + 2 * j + 1 for j in range(HD // 2))
    return np.array(idx)


def _prepare_in_maps(inputs):
    x = np.ascontiguousarray(np.asarray(inputs["x"], dtype=np.float32))
    fc = np.asarray(inputs["freqs_cos"], dtype=np.float32)
    fs = np.asarray(inputs["freqs_sin"], dtype=np.float32)
    wq = np.asarray(inputs["wq"], dtype=np.float32)
    wk = np.asarray(inputs["wk"], dtype=np.float32)
    wv = np.asarray(inputs["wv"], dtype=np.float32)
    wo = np.asarray(inputs["wo"], dtype=np.float32)
    aq = np.asarray(inputs["aq"], dtype=np.float32)
    bq = np.asarray(inputs["bq"], dtype=np.float32)
    ak = np.asarray(inputs["ak"], dtype=np.float32)
    bk = np.asarray(inputs["bk"], dtype=np.float32)
    av = np.asarray(inputs["av"], dtype=np.float32)
    bv = np.asarray(inputs["bv"], dtype=np.float32)
    ao = np.asarray(inputs["ao"], dtype=np.float32)
    bo = np.asarray(inputs["bo"], dtype=np.float32)

    permQ = _rope_perm(HPC)
    permK = _rope_perm(KVPC)
    # fold LoRA into dense weights: W_eff = W + B @ A
    wq = wq + bq.astype(np.float64) @ aq.astype(np.float64)
    wk = wk + bk.astype(np.float64) @ ak.astype(np.float64)
    wv = wv + bv.astype(np.float64) @ av.astype(np.float64)
    wo = wo + bo.astype(np.float64) @ ao.astype(np.float64)
    wq = wq.astype(np.float32)
    wk = wk.astype(np.float32)
    wv = wv.astype(np.float32)
    wo = wo.astype(np.float32)
    fcT = np.ascontiguousarray(fc.T)                       # [64, S]
    fsT = np.ascontiguousarray(fs.T)
    cs = np.concatenate([fcT, fcT], axis=0)                # [128, S]
    sn = np.concatenate([fsT, fsT], axis=0)
    rt = np.zeros((HD, HD), np.float32)
    for j in range(HD // 2):
        rt[j, 64 + j] = 1.0      # (R^T)[j, 64+j] = R[64+j, j] = +1
        rt[64 + j, j] = -1.0     # (R^T)[64+j, j] = R[j, 64+j] = -1
    ones = np.ones((HD, HD), np.float32)
    kk = np.arange(HD)[:, None]
    qq = np.arange(SC)[None, :]
    mk = np.stack([(qq >= (128 * r + kk)).astype(np.float32) for r in range(4)],
                  axis=1)                                  # [128, 4, SC]

    xt_cache = {}
    in_maps = []
    for c in range(8):
        b, g = c // 2, c % 2
        if b not in xt_cache:
            xt_cache[b] = np.ascontiguousarray(x[b].T)
        fq = slice(g * FQ, (g + 1) * FQ)
        fkv = slice(g * FKV, (g + 1) * FKV)
        wq_g = wq[fq][permQ]
        wk_g = wk[fkv][permK]
        in_maps.append({
            "xt": xt_cache[b],
            "wq": np.ascontiguousarray(wq_g.T),
            "wk": np.ascontiguousarray(wk_g.T),
            "wv": np.ascontiguousarray(wv[fkv].T),
            "wo": np.ascontiguousarray(wo[:, fq].T),
            "cs": cs, "sn": sn, "rt": rt, "on": ones, "mk": mk,
        })
    return in_maps


def _get_program():
    if "nc" not in _cache:
        _cache["nc"] = _build_program()
    return _cache["nc"]


def run(inputs, trace=False):
    from concourse import bass_utils
    nc = _get_program()
    in_maps = _prepare_in_maps(inputs)
    res = bass_utils.run_bass_kernel_spmd(
        nc, in_maps, list(range(8)), trace=trace)
    ys = [res.results[c]["y"] for c in range(8)]
    out = np.empty((B, S, DIM), np.float32)
    for b in range(B):
        out[b] = ys[2 * b] + ys[2 * b + 1]
    return out, res


def kernel(**inputs):
    out, _ = run(inputs, trace=False)
    return out


def bench(inputs, iters=20, n_cores=8):
    """Time repeated NEFF executions with device-resident inputs.

    Mirrors bass2jax.run_bass_via_pjrt's multi-core path without donation so
    buffers can be reused across calls.  Returns (avg_exec_seconds, output).
    """
    import time

    import jax
    import concourse.mybir as mybir
    from concourse import bass2jax
    from concourse.bass2jax import _bass_exec_p, partition_id_tensor
    from jax.sharding import Mesh, NamedSharding, PartitionSpec

    bass2jax.install_neuronx_cc_hook()
    nc = _get_program()
    in_maps = _prepare_in_maps(inputs)

    partition_name = nc.partition_id_tensor.name if nc.partition_id_tensor else None
    in_names, out_names, out_avals = [], [], []
    for alloc in nc.m.functions[0].allocations:
        if not isinstance(alloc, mybir.MemoryLocationSet):
            continue
        name = alloc.memorylocations[0].name
        if alloc.kind == "ExternalInput":
            if name != partition_name:
                in_names.append(name)
        elif alloc.kind == "ExternalOutput":
            out_names.append(name)
            out_avals.append(jax.core.ShapedArray(
                tuple(alloc.tensor_shape), mybir.dt.np(alloc.dtype)))
    n_params = len(in_names)
    all_names = list(in_names) + out_names
    if partition_name is not None:
        all_names.append(partition_name)

    def _body(*args):
        operands = list(args)
        if partition_name is not None:
            operands.append(partition_id_tensor())
        outs = _bass_exec_p.bind(
            *operands,
            out_avals=tuple(out_avals),
            in_names=tuple(all_names),
            out_names=tuple(out_names),
            lowering_input_output_aliases=(),
            sim_require_finite=True,
            sim_require_nnan=True,
            nc=nc,
        )
        return tuple(outs)

    devices = jax.devices()[:n_cores]
    mesh = Mesh(np.asarray(devices), ("core",))
    spec = NamedSharding(mesh, PartitionSpec("core"))
    from jax.experimental.shard_map import shard_map
    sharded = jax.jit(shard_map(
        _body, mesh=mesh,
        in_specs=(PartitionSpec("core"),) * (n_params + len(out_names)),
        out_specs=(PartitionSpec("core"),) * len(out_names),
        check_rep=False), keep_unused=True)

    concat_in = [
        jax.device_put(
            np.concatenate([np.asarray(in_maps[c][nm]) for c in range(n_cores)],
                           axis=0), spec)
        for nm in in_names]
    concat_zeros = [
        jax.device_put(
            np.zeros((n_cores * a.shape[0], *a.shape[1:]), a.dtype), spec)
        for a in out_avals]
    out = sharded(*concat_in, *concat_zeros)
    jax.block_until_ready(out)
    t0 = time.perf_counter()
    for _ in range(iters):
        out = sharded(*concat_in, *concat_zeros)
    jax.block_until_ready(out)
    t1 = time.perf_counter()

    if n_cores != 8:
        return (t1 - t0) / iters, None
    ys = np.asarray(out[out_names.index("y")]).reshape(n_cores, S, DIM)
    full = np.empty((B, S, DIM), np.float32)
    for b in range(B):
        full[b] = ys[2 * b] + ys[2 * b + 1]
    return (t1 - t0) / iters, full





# revision 29
# speedup vs baseline: 3.8454x; 3.8454x over previous
"""Trainium2 Bass kernel for AttentionWithSharedWeights (LoRA attention, GQA, RoPE).

Sharding over 8 NeuronCores: batch (4) x head-group (2).  Each core computes
8 Q heads / 2 KV heads of one batch and a head-sliced partial of the output
projection; the host sums the two partials per batch.

v2: fully fused fp16 pipeline.  One pass per 512-token chunk runs
  A(sc):  QKV projections + RoPE from resident fp16 weights
  B(qc=sc): causal attention for q-chunk sc against all k-tiles <= sc
  C(qc=sc): output projection for the chunk
with the Tile scheduler interleaving phases across chunks (PE fills
attention-dependency stalls with projection matmuls of the next chunk).

Key differences vs v1:
  - All matmul operands fp16 (1 cycle/row on PE, same as f32r, but half the
    SBUF/DMA and 4x DVE element rate); PSUM accumulation stays fp32.
  - No DRAM spills: Q and attention outputs stay resident in SBUF.
  - Softmax denominator via a fp16 running-sum chain on DVE + ONE ones-matmul
    per (head, q-chunk) instead of one per k-tile (removes ~12% of PE work).
  - Host pre-lays weights in the exact SBUF layout (contiguous DMA lines).
"""

import numpy as np

B, S, DIM = 4, 2048, 2048
NH, NKV, HD = 16, 4, 128
LR = 16          # lora rank
SC = 512         # sequence chunk
NSC = S // SC    # 4
NKT = S // HD    # 16 k-tiles
HPC = NH // 2    # 8 q heads per core
KVPC = NKV // 2  # 2 kv heads per core
FQ = HPC * HD    # 1024 q features per core
FKV = KVPC * HD  # 256 kv features per core
SCALE = 1.0 / float(np.sqrt(HD))

_cache = {}


def _build_program(repeat=1):
    import concourse.mybir as mybir
    import concourse.tile as tile
    from concourse import bacc

    f16 = mybir.dt.float16
    f32 = mybir.dt.float32
    Exp = mybir.ActivationFunctionType.Exp

    nc = bacc.Bacc()

    # ---- DRAM parameters (per-core views, host-prepared layouts) ----
    xt_d = nc.declare_dram_parameter("xt", [DIM, S], f16, isOutput=False)
    # weights already in SBUF layout; wq has ft outermost so each per-ft DMA
    # is one contiguous block
    wq_d = nc.declare_dram_parameter("wq", [HPC, HD, NKT, HD], f16, isOutput=False)
    wk_d = nc.declare_dram_parameter("wk", [HD, NKT, FKV], f16, isOutput=False)
    wv_d = nc.declare_dram_parameter("wv", [HD, NKT, FKV], f16, isOutput=False)
    wo_d = nc.declare_dram_parameter("wo", [FQ, DIM], f16, isOutput=False)
    cs_d = nc.declare_dram_parameter("cs", [HD, S], f16, isOutput=False)
    sn_d = nc.declare_dram_parameter("sn", [HD, S], f16, isOutput=False)
    rt_d = nc.declare_dram_parameter("rt", [HD, HD], f16, isOutput=False)
    on_d = nc.declare_dram_parameter("on", [HD, HD], f16, isOutput=False)
    mk_d = nc.declare_dram_parameter("mk", [HD, HD], f16, isOutput=False)
    y_d = nc.declare_dram_parameter("y", [S, DIM], f16, isOutput=True)

    with tile.TileContext(nc) as tc:
        # persistent SBUF tensors
        kt_t = nc.alloc_sbuf_tensor("kt_res", [HD, KVPC, S], f16)
        v_t = nc.alloc_sbuf_tensor("v_res", [HD, NKT, FKV], f16)
        kt_sb = kt_t[:]       # K feat-major, roped
        v_sb = v_t[:]         # V token-major

        with tc.tile_pool(name="pw", bufs=1) as pw, \
             tc.tile_pool(name="px", bufs=2) as px, \
             tc.tile_pool(name="pq", bufs=2) as pq, \
             tc.tile_pool(name="pot", bufs=2) as pot, \
             tc.tile_pool(name="pr", bufs=2) as pr, \
             tc.tile_pool(name="pe", bufs=6) as pe, \
             tc.tile_pool(name="pes", bufs=2) as pes, \
             tc.tile_pool(name="pinv", bufs=2) as pinv, \
             tc.tile_pool(name="py", bufs=3) as py, \
             tc.tile_pool(name="aps", bufs=2, space="PSUM") as aps, \
             tc.tile_pool(name="sps", bufs=2, space="PSUM") as sps, \
             tc.tile_pool(name="ops", bufs=2, space="PSUM") as ops, \
             tc.tile_pool(name="yps", bufs=2, space="PSUM") as yps:

            # DMA order: first-needed data first.  wq ft0/ft1 + x chunk 0
            # unblock the first projection group within a few us.
            cs_sb = pw.tile([HD, S], f16)
            sn_sb = pw.tile([HD, S], f16)
            rt_sb = pw.tile([HD, HD], f16)
            on_sb = pw.tile([HD, HD], f16)
            mk_sb = pw.tile([HD, HD], f16)
            wq_sb = pw.tile([HD, NKT, FQ], f16)
            wk_sb = pw.tile([HD, NKT, FKV], f16)
            wv_sb = pw.tile([HD, NKT, FKV], f16)

            def load_wq(ft):
                nc.sync.dma_start(
                    wq_sb[:, :, ft * HD:(ft + 1) * HD], wq_d[ft])

            # compute starts with K then V then Q heads: match that order
            nc.sync.dma_start(wk_sb[:], wk_d[:])
            nc.sync.dma_start(wv_sb[:], wv_d[:])

            xcs = {}

            def fetch_x(g):
                xc = px.tile([HD, NKT, SC], f16, tag="xc")
                sc = g % NSC
                ssl = slice(sc * SC, (sc + 1) * SC)
                for kt in range(NKT):
                    nc.sync.dma_start(
                        xc[:, kt, :], xt_d[kt * HD:(kt + 1) * HD, ssl])
                xcs[g] = xc

            fetch_x(0)

            nc.sync.dma_start(cs_sb[:], cs_d[:])
            nc.sync.dma_start(sn_sb[:], sn_d[:])
            nc.sync.dma_start(rt_sb[:], rt_d[:])
            nc.sync.dma_start(on_sb[:], on_d[:])
            nc.sync.dma_start(mk_sb[:], mk_d[:])
            for ft in range(HPC):
                load_wq(ft)

            # wo needed only from C(0) (~25% in); spread its load
            wo_sb = pw.tile([HD, HPC, DIM], f16)
            for h in range(HPC):
                nc.sync.dma_start(
                    wo_sb[:, h, :], wo_d[h * HD:(h + 1) * HD, :])

            for g in range(repeat * NSC):
                    rep, sc = divmod(g, NSC)
                    ssl = slice(sc * SC, (sc + 1) * SC)
                    xc = xcs.pop(g)
                    # prefetch next chunk's x
                    if g + 1 < repeat * NSC:
                        fetch_x(g + 1)

                    cs_sl = cs_sb[:, ssl]
                    sn_sl = sn_sb[:, ssl]

                    def rope_start(raw_ps):
                        """Evict pre-rope PSUM to fp16 SBUF (ACT); PE part
                        is emitted later (rope_finish) so the engine FIFO
                        never head-of-line blocks on the eviction."""
                        raw = pr.tile([HD, SC], f16, tag="rope_raw")
                        nc.scalar.copy(out=raw[:], in_=raw_ps[:])
                        return raw

                    def rope_finish(raw, out_ap):
                        rot_ps = aps.tile([HD, SC], f32, tag="a512")
                        nc.tensor.matmul(rot_ps[:], rt_sb[:], raw[:],
                                         start=True, stop=True)
                        tmp = pr.tile([HD, SC], f16, tag="rope_tmp")
                        nc.gpsimd.tensor_mul(tmp[:], raw[:], cs_sl)
                        e1 = pr.tile([HD, SC], f16, tag="rope_e1")
                        nc.vector.tensor_mul(e1[:], rot_ps[:], sn_sl)
                        nc.vector.tensor_add(out_ap, tmp[:], e1[:])

                    # ---------------- A: projections + RoPE ----------------
                    # K and V first so attention on this chunk can start
                    # while the Q heads are still projecting.
                    q_sb = pq.tile([HD, HPC, SC], f16, tag="q")
                    pending = [None]

                    def proj_block(w_sl, out_ap):
                        q_ps = aps.tile([HD, SC], f32, tag="a512")
                        for kt in range(NKT):
                            nc.tensor.matmul(q_ps[:], w_sl[:, kt, :],
                                             xc[:, kt, :],
                                             start=(kt == 0),
                                             stop=(kt == NKT - 1))
                        raw = rope_start(q_ps)
                        if pending[0] is not None:
                            rope_finish(*pending[0])
                        pending[0] = (raw, out_ap)

                    for kf in range(KVPC):
                        proj_block(wk_sb[:, :, kf * HD:(kf + 1) * HD],
                                   kt_sb[:, kf, ssl])

                    # V: token-major, two 128-token groups per PSUM tile
                    for vp in range(SC // HD // 2):
                        v_ps = aps.tile([HD, SC], f32, tag="a512")
                        for i in range(2):
                            st = 2 * vp + i
                            tsl = slice(st * HD, (st + 1) * HD)
                            for kt in range(NKT):
                                nc.tensor.matmul(
                                    v_ps[:, i * FKV:(i + 1) * FKV],
                                    xc[:, kt, tsl], wv_sb[:, kt, :],
                                    start=(kt == 0), stop=(kt == NKT - 1),
                                    skip_group_check=True)
                        base = sc * (SC // HD) + 2 * vp
                        nc.scalar.copy(out=v_sb[:, base:base + 2, :],
                                       in_=v_ps[:])
                        if pending[0] is not None:
                            rope_finish(*pending[0])
                            pending[0] = None

                    # ------- B setup: attention emitted per-head, woven in
                    # between the Q projection blocks so every engine's FIFO
                    # alternates projection/attention work.
                    qc = sc
                    nkt = 4 * qc + 4
                    ot_sb = pot.tile([HD, HPC, SC], f16, tag="ot")

                    def normalize(esum, ot_ps, h):
                        """Denominator matmul + 1/sum rescale for head h."""
                        bc_ps = sps.tile([HD, SC], f32, tag="s")
                        nc.tensor.matmul(bc_ps[:], on_sb[:], esum[:],
                                         start=True, stop=True)
                        inv = pinv.tile([HD, SC], f32, tag="inv")
                        nc.vector.reciprocal(inv[:], bc_ps[:])
                        nc.vector.tensor_mul(ot_sb[:, h, :], ot_ps[:], inv[:])

                    pend_n = [None]

                    def attn_head(h):
                        kv = h // (HPC // KVPC)
                        qh = q_sb[:, h, :]
                        ot_ps = ops.tile([HD, SC], f32, tag="o")
                        esum = pes.tile([HD, SC], f16, tag="es")
                        for kt in range(nkt):
                            r = kt - 4 * qc
                            q0 = max(r, 0) * HD
                            s_ps = sps.tile([HD, SC], f32, tag="s")
                            nc.tensor.matmul(
                                s_ps[:, q0:],
                                kt_sb[:, kv, kt * HD:(kt + 1) * HD],
                                qh[:, q0:], start=True, stop=True)
                            e = pe.tile([HD, SC], f16, tag="e")
                            nc.scalar.activation(e[:, q0:], s_ps[:, q0:],
                                                 Exp, scale=SCALE)
                            if r >= 0:
                                # intra-tile causal triangle mask (in place)
                                nc.gpsimd.tensor_mul(
                                    e[:, q0:q0 + HD], e[:, q0:q0 + HD],
                                    mk_sb[:])
                            nc.tensor.matmul(
                                ot_ps[:, q0:],
                                v_sb[:, kt, kv * HD:(kv + 1) * HD],
                                e[:, q0:], start=(kt == 0),
                                stop=(kt == nkt - 1), skip_group_check=True)
                            if kt == 0:
                                nc.vector.tensor_copy(out=esum[:], in_=e[:])
                            else:
                                nc.vector.tensor_add(esum[:, q0:],
                                                     esum[:, q0:], e[:, q0:])
                            if kt == 1 and pend_n[0] is not None:
                                normalize(*pend_n[0])
                                pend_n[0] = None
                        pend_n[0] = (esum, ot_ps, h)

                    def emit_c_groups(n):
                        """Emit up to n pending output-projection groups of
                        the PREVIOUS chunk (C is delayed one chunk so its PE
                        work fills attention-dependency gaps, most
                        importantly during the last chunk's attention)."""
                        while pend_c and n > 0:
                            ot_prev, qcp, dc, qs = pend_c.pop(0)
                            dsl = slice(dc * SC, (dc + 1) * SC)
                            qt0 = qcp * SC + qs * HD
                            y_ps = yps.tile([HD, SC], f32, tag="y")
                            for ft in range(HPC):
                                nc.tensor.matmul(
                                    y_ps[:],
                                    ot_prev[:, ft, qs * HD:(qs + 1) * HD],
                                    wo_sb[:, ft, dsl],
                                    start=(ft == 0), stop=(ft == HPC - 1))
                            y_sb = py.tile([HD, SC], f16, tag="ysb")
                            nc.scalar.copy(out=y_sb[:], in_=y_ps[:])
                            nc.sync.dma_start(y_d[qt0:qt0 + HD, dsl], y_sb[:])
                            n -= 1

                    for ft in range(HPC):
                        proj_block(wq_sb[:, :, ft * HD:(ft + 1) * HD],
                                   q_sb[:, ft, :])
                        if ft >= 1:
                            attn_head(ft - 1)
                            emit_c_groups(2)
                    rope_finish(*pending[0])
                    pending[0] = None
                    attn_head(HPC - 1)
                    emit_c_groups(2)
                    normalize(*pend_n[0])
                    pend_n[0] = None
                    emit_c_groups(len(pend_c))
                    pend_c.extend(
                        (ot_sb, qc, dc, qs)
                        for dc in range(DIM // SC)
                        for qs in range(SC // HD))

            emit_c_groups_final = pend_c
            # final chunk's output projection
            for ot_prev, qcp, dc, qs in emit_c_groups_final:
                dsl = slice(dc * SC, (dc + 1) * SC)
                qt0 = qcp * SC + qs * HD
                y_ps = yps.tile([HD, SC], f32, tag="y")
                for ft in range(HPC):
                    nc.tensor.matmul(
                        y_ps[:], ot_prev[:, ft, qs * HD:(qs + 1) * HD],
                        wo_sb[:, ft, dsl],
                        start=(ft == 0), stop=(ft == HPC - 1))
                y_sb = py.tile([HD, SC], f16, tag="ysb")
                nc.scalar.copy(out=y_sb[:], in_=y_ps[:])
                nc.sync.dma_start(y_d[qt0:qt0 + HD, dsl], y_sb[:])

    nc.finalize()
    return nc


def _rope_perm(nheads):
    """Row permutation putting even dims first within each head."""
    idx = []
    for h in range(nheads):
        base = h * HD
        idx.extend(base + 2 * j for j in range(HD // 2))
        idx.extend(base + 2 * j + 1 for j in range(HD // 2))
    return np.array(idx)


def _prepare_in_maps(inputs):
    x = np.asarray(inputs["x"], dtype=np.float32)
    fc = np.asarray(inputs["freqs_cos"], dtype=np.float32)
    fs = np.asarray(inputs["freqs_sin"], dtype=np.float32)
    wq = np.asarray(inputs["wq"], dtype=np.float32)
    wk = np.asarray(inputs["wk"], dtype=np.float32)
    wv = np.asarray(inputs["wv"], dtype=np.float32)
    wo = np.asarray(inputs["wo"], dtype=np.float32)
    aq = np.asarray(inputs["aq"], dtype=np.float32)
    bq = np.asarray(inputs["bq"], dtype=np.float32)
    ak = np.asarray(inputs["ak"], dtype=np.float32)
    bk = np.asarray(inputs["bk"], dtype=np.float32)
    av = np.asarray(inputs["av"], dtype=np.float32)
    bv = np.asarray(inputs["bv"], dtype=np.float32)
    ao = np.asarray(inputs["ao"], dtype=np.float32)
    bo = np.asarray(inputs["bo"], dtype=np.float32)

    permQ = _rope_perm(HPC)
    permK = _rope_perm(KVPC)
    # fold LoRA into dense weights: W_eff = W + B @ A
    wq = (wq + bq.astype(np.float64) @ aq.astype(np.float64)).astype(np.float32)
    wk = (wk + bk.astype(np.float64) @ ak.astype(np.float64)).astype(np.float32)
    wv = (wv + bv.astype(np.float64) @ av.astype(np.float64)).astype(np.float32)
    wo = (wo + bo.astype(np.float64) @ ao.astype(np.float64)).astype(np.float32)
    fcT = fc.T                                             # [64, S]
    fsT = fs.T
    cs = np.concatenate([fcT, fcT], axis=0).astype(np.float16)  # [128, S]
    sn = np.concatenate([fsT, fsT], axis=0).astype(np.float16)
    rt = np.zeros((HD, HD), np.float16)
    for j in range(HD // 2):
        rt[j, 64 + j] = 1.0      # (R^T)[j, 64+j] = R[64+j, j] = +1
        rt[64 + j, j] = -1.0     # (R^T)[64+j, j] = R[j, 64+j] = -1
    ones = np.ones((HD, HD), np.float16)
    kk = np.arange(HD)[:, None]
    qq = np.arange(HD)[None, :]
    mk = (qq >= kk).astype(np.float16)                     # [128, 128]

    def sbuf_layout(w):
        # [DIM, F] -> [128, DIM//128, F] partition-major contiguous
        f = w.shape[1]
        return np.ascontiguousarray(
            w.reshape(NKT, HD, f).transpose(1, 0, 2)).astype(np.float16)

    def sbuf_layout_ft(w):
        # [DIM, FQ] -> [HPC, 128, DIM//128, 128]: ft-outer contiguous blocks
        return np.ascontiguousarray(
            w.reshape(NKT, HD, HPC, HD).transpose(2, 1, 0, 3)).astype(np.float16)

    xt_cache = {}
    in_maps = []
    for c in range(8):
        b, g = c // 2, c % 2
        if b not in xt_cache:
            xt_cache[b] = np.ascontiguousarray(x[b].T).astype(np.float16)
        fq = slice(g * FQ, (g + 1) * FQ)
        fkv = slice(g * FKV, (g + 1) * FKV)
        wq_g = wq[fq][permQ]
        wk_g = wk[fkv][permK]
        in_maps.append({
            "xt": xt_cache[b],
            "wq": sbuf_layout_ft(np.ascontiguousarray(wq_g.T)),
            "wk": sbuf_layout(np.ascontiguousarray(wk_g.T)),
            "wv": sbuf_layout(np.ascontiguousarray(wv[fkv].T)),
            "wo": np.ascontiguousarray(wo[:, fq].T).astype(np.float16),
            "cs": cs, "sn": sn, "rt": rt, "on": ones, "mk": mk,
        })
    return in_maps


def _get_program(repeat=1):
    key = ("nc", repeat)
    if key not in _cache:
        _cache[key] = _build_program(repeat)
    return _cache[key]


def run(inputs, trace=False):
    from concourse import bass_utils
    nc = _get_program()
    in_maps = _prepare_in_maps(inputs)
    res = bass_utils.run_bass_kernel_spmd(
        nc, in_maps, list(range(8)), trace=trace)
    ys = [res.results[c]["y"] for c in range(8)]
    out = np.empty((B, S, DIM), np.float32)
    for b in range(B):
        out[b] = ys[2 * b].astype(np.float32) + ys[2 * b + 1].astype(np.float32)
    return out, res


def kernel(**inputs):
    out, _ = run(inputs, trace=False)
    return out


def make_runner(inputs, repeat=1, n_cores=8):
    """Build a zero-arg callable executing the repeat-x NEFF once (blocking),
    plus a callable fetching the full output.  Device-resident inputs."""
    import jax
    import concourse.mybir as mybir
    from concourse import bass2jax
    from concourse.bass2jax import _bass_exec_p, partition_id_tensor
    from jax.sharding import Mesh, NamedSharding, PartitionSpec

    bass2jax.install_neuronx_cc_hook()
    nc = _get_program(repeat)
    in_maps = _prepare_in_maps(inputs)

    partition_name = nc.partition_id_tensor.name if nc.partition_id_tensor else None
    in_names, out_names, out_avals = [], [], []
    for alloc in nc.m.functions[0].allocations:
        if not isinstance(alloc, mybir.MemoryLocationSet):
            continue
        name = alloc.memorylocations[0].name
        if alloc.kind == "ExternalInput":
            if name != partition_name:
                in_names.append(name)
        elif alloc.kind == "ExternalOutput":
            out_names.append(name)
            out_avals.append(jax.core.ShapedArray(
                tuple(alloc.tensor_shape), mybir.dt.np(alloc.dtype)))
    n_params = len(in_names)
    all_names = list(in_names) + out_names
    if partition_name is not None:
        all_names.append(partition_name)

    def _body(*args):
        operands = list(args)
        if partition_name is not None:
            operands.append(partition_id_tensor())
        outs = _bass_exec_p.bind(
            *operands,
            out_avals=tuple(out_avals),
            in_names=tuple(all_names),
            out_names=tuple(out_names),
            lowering_input_output_aliases=(),
            sim_require_finite=True,
            sim_require_nnan=True,
            nc=nc,
        )
        return tuple(outs)

    devices = jax.devices()[:n_cores]
    mesh = Mesh(np.asarray(devices), ("core",))
    spec = NamedSharding(mesh, PartitionSpec("core"))
    from jax.experimental.shard_map import shard_map
    sharded = jax.jit(shard_map(
        _body, mesh=mesh,
        in_specs=(PartitionSpec("core"),) * (n_params + len(out_names)),
        out_specs=(PartitionSpec("core"),) * len(out_names),
        check_rep=False), keep_unused=True)

    concat_in = [
        jax.device_put(
            np.concatenate([np.asarray(in_maps[c][nm]) for c in range(n_cores)],
                           axis=0), spec)
        for nm in in_names]
    concat_zeros = [
        jax.device_put(
            np.zeros((n_cores * a.shape[0], *a.shape[1:]), a.dtype), spec)
        for a in out_avals]

    state = {}

    def run_once():
        out = sharded(*concat_in, *concat_zeros)
        jax.block_until_ready(out)
        state["out"] = out

    def fetch():
        ys = np.asarray(state["out"][out_names.index("y")]).reshape(
            n_cores, S, DIM)
        full = np.empty((B, S, DIM), np.float32)
        for b in range(B):
            full[b] = (ys[2 * b].astype(np.float32)
                       + ys[2 * b + 1].astype(np.float32))
        return full

    return run_once, fetch


def bench(inputs, iters=20, n_cores=8, repeat=1):
    """Back-compat: average seconds per call + output."""
    import time
    run_once, fetch = make_runner(inputs, repeat=repeat, n_cores=n_cores)
    run_once()
    t0 = time.perf_counter()
    for _ in range(iters):
        run_once()
    t1 = time.perf_counter()
    return (t1 - t0) / iters, fetch()
